# revision 22
# baseline (speedup 1.0000x reference)
"""Attention pooling kernel for Trainium2 (8 NeuronCores).

Reference computation (per batch b):
    score   = tanh(x @ W + b)          # (S, 1)
    weights = softmax(score, axis=seq) # (S, 1)
    context = sum(x * weights, axis=seq)  # (D,)

Sharding: data-parallel over batch (32 batches -> 4 per core).

Strategy per core (compute in bf16 on the TensorEngine, f32 accumulation):
  - Host converts x to bf16 and ships BOTH layouts (seq-major for the
    context matmul, dim-major for the score matmul) pre-swizzled into
    128-partition tiles so every load is a fully contiguous DMA.
  - score: lhsT = W d-chunk [128,1], rhs = xT [128d, 512s] -> psum [1, 512]
    accumulated over 4 d-chunks -> score lives as [1, S] on partition 0.
  - softmax entirely in the [1, S] layout: tanh is bounded so no
    max-subtraction is needed; exp uses activation accum_out for the sum,
    so no cross-partition reduce is needed either.
  - weights output is written straight from the f32 [1, S] tile.
  - bf16 weights roundtrip through DRAM + xbar-transpose DMA to land as
    [128, 32] (seq on partitions) = the lhsT columns for the context matmul.
  - context: lhsT = weight column [128,1], rhs = x_native [128s, 512d]
    -> psum [1, 512] accumulated over 32 s-tiles.
  - PE instructions are hardware-decoded and only carry a single sync wait:
    tiny "absorber" matmuls take the DMA-completion waits so every real
    matmul needs at most one.
"""

import sys

for p in ("/opt/trn_rl_repo",):
    if p not in sys.path:
        sys.path.insert(0, p)

import numpy as np
import ml_dtypes

B, S, D = 32, 4096, 512
NCORES = 8
BPC = B // NCORES  # batches per core
ST = S // 128      # 32 seq tiles
DC = D // 128      # 4 dim chunks

_cache = {}


def _build(fix_waits=True):
    import concourse.bass as bass
    import concourse.mybir as mybir
    from concourse import tile
    from concourse.tile_rust import add_dep_helper

    dt = mybir.dt
    AF = mybir.ActivationFunctionType

    nc = bass.Bass()
    # xt: dim-major halves, [b, w, p, c, 2048]: window w covers seq
    #     [w*2048, (w+1)*2048); element (p, c, s) = x[b, w*2048+s, c*128+p]
    # xn: seq-major halves, [b, g, p, t, 512]: element (p, t, d) =
    #     x[b, (g*16+t)*128 + p, d]
    xt = nc.declare_dram_parameter(
        "xt", [BPC, 2, 128, DC, S // 2], dt.bfloat16, isOutput=False
    )
    xn = nc.declare_dram_parameter(
        "xn", [BPC, 2, 128, ST // 2, D], dt.bfloat16, isOutput=False
    )
    wv = nc.declare_dram_parameter("wv", [128, DC], dt.bfloat16, isOutput=False)
    bt = nc.declare_dram_parameter("bt", [128, ST], dt.float32, isOutput=False)
    ident = nc.declare_dram_parameter("ident", [128, 128], dt.float32, isOutput=False)
    ones_c = nc.declare_dram_parameter("ones_c", [128, 1], dt.float32, isOutput=False)
    ones_r = nc.declare_dram_parameter("ones_r", [1, 128], dt.float32, isOutput=False)
    ctx_out = nc.declare_dram_parameter("ctx_out", [BPC, D], dt.float32, isOutput=True)
    w_out = nc.declare_dram_parameter("w_out", [BPC, S], dt.float32, isOutput=True)

    SH = S // 2  # 2048, seq window size

    with tile.TileContext(nc) as tc:
        with (
            tc.tile_pool(name="xtpool", bufs=4) as xtpool,
            tc.tile_pool(name="xnpool", bufs=4) as xnpool,
            tc.tile_pool(name="consts", bufs=1) as cpool,
            tc.tile_pool(name="small", bufs=2) as spool,
            tc.tile_pool(name="psum_sc", bufs=2, space="PSUM") as psc,
            tc.tile_pool(name="psum_ctx", bufs=2, space="PSUM") as pctx,
            tc.tile_pool(name="psum_m1", bufs=1, space="PSUM") as pm1,
            tc.tile_pool(name="psum_wt", bufs=1, space="PSUM") as pwtp,
            tc.tile_pool(name="dram", bufs=2, space="DRAM") as dpool,
        ):
            wv_sb = cpool.tile([128, DC], dt.bfloat16, tag="wv")
            nc.sync.dma_start(out=wv_sb[:], in_=wv[:])
            bt_sb = cpool.tile([128, ST], dt.float32, tag="bt")
            nc.sync.dma_start(out=bt_sb[:], in_=bt[:])
            id_sb = cpool.tile([128, 128], dt.float32, tag="id")
            nc.sync.dma_start(out=id_sb[:], in_=ident[:])
            oc_sb = cpool.tile([128, 1], dt.float32, tag="oc")
            nc.sync.dma_start(out=oc_sb[:], in_=ones_c[:])
            or_sb = cpool.tile([1, 128], dt.float32, tag="or")
            nc.sync.dma_start(out=or_sb[:], in_=ones_r[:])

            # ~5us of junk matmuls during the load lead-in keeps the PE
            # HAM clock-gate warm for the first real score burst
            warm_ps = pm1.tile([1, 128], dt.float32, tag="m1")
            for _ in range(18):
                nc.tensor.matmul(
                    warm_ps[:], id_sb[:, 0:1], id_sb[:], start=True, stop=True
                )

            state = {}

            def stage_score(b):
                xt_sb, xn_sb = [], []
                for w in range(2):
                    t = xtpool.tile([128, DC * SH], dt.bfloat16, tag="xtw")
                    nc.sync.dma_start(
                        out=t[:], in_=xt[b, w].rearrange("p c s -> p (c s)")
                    )
                    xt_sb.append(t)
                for g in range(2):
                    t = xnpool.tile([128, (ST // 2) * D], dt.bfloat16, tag="xnw")
                    nc.sync.dma_start(
                        out=t[:], in_=xn[b, g].rearrange("p t d -> p (t d)")
                    )
                    xn_sb.append(t)

                # score = x @ W in [1, S] (bf16), via M=1 matmuls
                sc_bf = spool.tile([1, S], dt.bfloat16, tag="sc")
                scT = spool.tile([128, ST], dt.bfloat16, tag="scT")
                for hh in range(2):
                    for q in range(2):
                        n = hh * 2 + q
                        so = q * 1024
                        ps = psc.tile([1, 1024], dt.float32, tag="ps")
                        for h in range(2):
                            for j in range(DC):
                                col = so + h * 512
                                mm = nc.tensor.matmul(
                                    ps[:, h * 512 : (h + 1) * 512],
                                    wv_sb[:, j : j + 1],
                                    xt_sb[hh][:, j * SH + col : j * SH + col + 512],
                                    start=(j == 0),
                                    stop=(j == DC - 1),
                                )
                        state[("last_mm", b)] = mm
                        nc.vector.tensor_copy(
                            sc_bf[:, n * 1024 : (n + 1) * 1024], ps[:]
                        )
                    # raw-score roundtrip: DRAM + xbar transpose, per half
                    wd = dpool.tile([ST // 2, 128], dt.bfloat16, tag=f"wd{hh}")
                    nc.gpsimd.dma_start(
                        out=wd[:].rearrange("a b -> (a b)").unsqueeze(0),
                        in_=sc_bf[:, hh * SH : (hh + 1) * SH],
                    )
                    nc.scalar.dma_start(
                        out=scT[:, hh * 16 : hh * 16 + 16], in_=wd[:], transpose=True
                    )
                state[b] = (xn_sb, scT)

            def stage_post(b):
                xn_sb, scT = state.pop(b)
                # softmax over seq, fully parallel in [128, ST] layout
                sraw = spool.tile([128, ST], dt.float32, tag="sraw")
                nc.vector.tensor_add(sraw[:], scT[:], bt_sb[:])
                nc.scalar.activation(sraw[:], sraw[:], AF.Tanh)
                esum = spool.tile([128, 1], dt.float32, tag="esum")
                nc.scalar.activation(sraw[:], sraw[:], AF.Exp, accum_out=esum[:])
                ps1 = pm1.tile([128, 1], dt.float32, tag="m1")
                mm1 = nc.tensor.matmul(
                    ps1[0:1, :], esum[:], oc_sb[:], start=True, stop=True
                )
                nxt = state.get(("last_mm", b + 1))
                if nxt is not None:
                    add_dep_helper(mm1.ins, nxt.ins, False, "pipeline order")
                rec = spool.tile([1, 1], dt.float32, tag="rec")
                nc.vector.reciprocal(rec[:], ps1[0:1, :])
                prb = pm1.tile([128, 1], dt.float32, tag="m1")
                nc.tensor.matmul(prb[:], or_sb[:], rec[:], start=True, stop=True)
                rbc = spool.tile([128, 1], dt.float32, tag="rbc")
                nc.scalar.copy(rbc[:], prb[:])
                w_f = spool.tile([128, ST], dt.float32, tag="wf")
                nc.scalar.mul(w_f[:], sraw[:], rbc[:])
                w_bf = spool.tile([128, ST], dt.bfloat16, tag="wbf")
                nc.vector.tensor_copy(w_bf[:], w_f[:])

                # context = sum_s w[s] * x[s, :]
                pc = pctx.tile([1, D], dt.float32, tag="pc")
                for i in range(ST):
                    nc.tensor.matmul(
                        pc[:],
                        w_bf[:, i : i + 1],
                        xn_sb[i // 16][:, (i % 16) * D : (i % 16 + 1) * D],
                        start=(i == 0),
                        stop=(i == ST - 1),
                    )
                ctx_sb = spool.tile([1, D], dt.float32, tag="ctx")
                nc.scalar.copy(ctx_sb[:], pc[:])
                nc.gpsimd.dma_start(out=ctx_out[b : b + 1, :], in_=ctx_sb[:])

                # weights out: PE transpose -> [32, 128] -> contiguous DRAM
                pwt = pwtp.tile([ST, 128], dt.float32, tag="wt")
                nc.tensor.transpose(pwt[:], w_f[:], id_sb[:])
                wt_sb = spool.tile([ST, 128], dt.float32, tag="wts")
                nc.scalar.copy(wt_sb[:], pwt[:])
                nc.gpsimd.dma_start(
                    out=w_out[b].rearrange("(a c) -> a c", c=128), in_=wt_sb[:]
                )

            # software pipeline: emit score(b+1) before post(b) so the PE
            # queue never head-of-line blocks on batch b's softmax
            stage_score(0)
            for b in range(BPC):
                if b + 1 < BPC:
                    stage_score(b + 1)
                stage_post(b)

    if fix_waits:
        _fix_pe_waits(nc, mybir)
    return nc


def _fix_pe_waits(nc, mybir):
    """Engine instructions hold a single hardware sync-wait slot; Tile
    sometimes emits 2+ waits on one instruction (psum/tile slot reuse), which
    walrus rejects with 'Too many sync wait commands'.  Splice standalone
    EventSemaphore instructions (one wait each) into the same engine queue
    immediately before each over-subscribed instruction — semantically
    identical, the sequencer just waits in two steps."""
    f = nc.m.functions[0]
    counter = [0]
    for blk in f.blocks:
        insts = list(blk.instructions)
        out = []
        changed = False
        for inst in insts:
            si = inst.sync_info
            nw = len(si.on_wait) if si is not None and si.on_wait else 0
            if nw > 1:
                waits = list(si.on_wait)
                for w in waits[:-1]:
                    es = mybir.InstEventSemaphore(
                        name=f"I-eswait-{counter[0]}", ins=[], outs=[]
                    )
                    counter[0] += 1
                    es.engine = inst.engine
                    es.sync_info = mybir.SyncInfo(on_wait=[w], on_update=[])
                    out.append(es)
                si.on_wait = waits[-1:]
                changed = True
            out.append(inst)
        if changed:
            blk.instructions = out


def _prep_inputs(x, W, b):
    bf16 = ml_dtypes.bfloat16
    xbf = x.astype(bf16)  # (B, S, D)
    # native, seq-major halves: xn[b, g, p, t, d] = x[b, (g*16+t)*128+p, d]
    xn = np.ascontiguousarray(
        xbf.reshape(B, 2, ST // 2, 128, D).transpose(0, 1, 3, 2, 4)
    )
    # dim-major halves: xt[b, w, p, c, s] = x[b, w*2048+s, c*128+p]
    xt = np.ascontiguousarray(
        xbf.reshape(B, 2, S // 2, DC, 128).transpose(0, 1, 4, 3, 2)
    )
    wv = np.ascontiguousarray(
        W.reshape(DC, 128).T.astype(bf16)
    )  # [128, DC], col j = W[j*128:(j+1)*128]
    bt = np.ascontiguousarray(b.reshape(ST, 128).T.astype(np.float32))
    ident = np.eye(128, dtype=np.float32)
    ones_c = np.ones((128, 1), dtype=np.float32)
    ones_r = np.ones((1, 128), dtype=np.float32)
    in_maps = []
    for c in range(NCORES):
        lo = c * BPC
        in_maps.append(
            {
                "xn": xn[lo : lo + BPC],
                "xt": xt[lo : lo + BPC],
                "wv": wv,
                "bt": bt,
                "ident": ident,
                "ones_c": ones_c,
                "ones_r": ones_r,
            }
        )
    return in_maps


def kernel(x, W, b):
    from concourse.bass_utils import run_bass_kernel_spmd

    x = np.asarray(x, dtype=np.float32)
    W = np.asarray(W, dtype=np.float32)
    b = np.asarray(b, dtype=np.float32)

    if "nc" not in _cache:
        _cache["nc"] = _build()
    nc = _cache["nc"]

    in_maps = _prep_inputs(x, W, b)
    res = run_bass_kernel_spmd(nc, in_maps, list(range(NCORES))).results

    context = np.concatenate(
        [np.asarray(res[c]["ctx_out"], dtype=np.float32) for c in range(NCORES)], axis=0
    )  # (B, D)
    weights = np.concatenate(
        [np.asarray(res[c]["w_out"], dtype=np.float32) for c in range(NCORES)], axis=0
    ).reshape(B, S, 1)
    return context, weights


# revision 23
# speedup vs baseline: 1.4922x; 1.4922x over previous
"""Attention pooling kernel for Trainium2 (8 NeuronCores).

Reference computation (per batch b):
    score   = tanh(x @ W + b)          # (S, 1)
    weights = softmax(score, axis=seq) # (S, 1)
    context = sum(x * weights, axis=seq)  # (D,)

Sharding: data-parallel over batch (32 batches -> 4 per core).

Strategy per core (compute in bf16 on the TensorEngine, f32 accumulation):
  - Host converts x to bf16 and ships BOTH layouts (seq-major for the
    context matmul, dim-major for the score matmul) pre-swizzled into
    128-partition tiles so every load is a fully contiguous DMA.
  - score: lhsT = W d-chunk [128,1], rhs = xT [128d, 512s] -> psum [1, 512]
    accumulated over 4 d-chunks -> score lives as [1, S] on partition 0.
  - softmax entirely in the [1, S] layout: tanh is bounded so no
    max-subtraction is needed; exp uses activation accum_out for the sum,
    so no cross-partition reduce is needed either.
  - weights output is written straight from the f32 [1, S] tile.
  - bf16 weights roundtrip through DRAM + xbar-transpose DMA to land as
    [128, 32] (seq on partitions) = the lhsT columns for the context matmul.
  - context: lhsT = weight column [128,1], rhs = x_native [128s, 512d]
    -> psum [1, 512] accumulated over 32 s-tiles.
  - PE instructions are hardware-decoded and only carry a single sync wait:
    tiny "absorber" matmuls take the DMA-completion waits so every real
    matmul needs at most one.
"""

import sys

for p in ("/opt/trn_rl_repo",):
    if p not in sys.path:
        sys.path.insert(0, p)

import numpy as np
import ml_dtypes

B, S, D = 32, 4096, 512
NCORES = 8
BPC = B // NCORES  # batches per core
ST = S // 128      # 32 seq tiles
DC = D // 128      # 4 dim chunks

_cache = {}


def _build(fix_waits=True):
    import concourse.bass as bass
    import concourse.mybir as mybir
    from concourse import tile
    from concourse.tile_rust import add_dep_helper

    dt = mybir.dt
    AF = mybir.ActivationFunctionType

    nc = bass.Bass()
    # xt: dim-major halves, [b, w, p, c, 2048]: window w covers seq
    #     [w*2048, (w+1)*2048); element (p, c, s) = x[b, w*2048+s, c*128+p]
    # xn: seq-major halves, [b, g, p, t, 512]: element (p, t, d) =
    #     x[b, (g*16+t)*128 + p, d]
    xt = nc.declare_dram_parameter(
        "xt", [BPC, 2, 128, DC, S // 2], dt.bfloat16, isOutput=False
    )
    xn = nc.declare_dram_parameter(
        "xn", [BPC, 2, 128, ST // 2, D], dt.bfloat16, isOutput=False
    )
    wv = nc.declare_dram_parameter("wv", [128, DC], dt.bfloat16, isOutput=False)
    bt = nc.declare_dram_parameter("bt", [128, ST], dt.float32, isOutput=False)
    ident = nc.declare_dram_parameter("ident", [128, 128], dt.float32, isOutput=False)
    ones_c = nc.declare_dram_parameter("ones_c", [128, 1], dt.float32, isOutput=False)
    ones_r = nc.declare_dram_parameter("ones_r", [1, 128], dt.float32, isOutput=False)
    ctx_out = nc.declare_dram_parameter("ctx_out", [BPC, D], dt.float32, isOutput=True)
    w_out = nc.declare_dram_parameter("w_out", [BPC, S], dt.float32, isOutput=True)

    SH = S // 2  # 2048, seq window size

    with tile.TileContext(nc) as tc:
        with (
            tc.tile_pool(name="xtpool", bufs=6) as xtpool,
            tc.tile_pool(name="xnpool", bufs=4) as xnpool,
            tc.tile_pool(name="consts", bufs=1) as cpool,
            tc.tile_pool(name="small", bufs=2) as spool,
            tc.tile_pool(name="psum_sc", bufs=2, space="PSUM") as psc,
            tc.tile_pool(name="psum_ctx", bufs=2, space="PSUM") as pctx,
            tc.tile_pool(name="psum_m1", bufs=1, space="PSUM") as pm1,
            tc.tile_pool(name="psum_wt", bufs=1, space="PSUM") as pwtp,
            tc.tile_pool(name="dram", bufs=2, space="DRAM") as dpool,
        ):
            wv_sb = cpool.tile([128, DC], dt.bfloat16, tag="wv")
            nc.sync.dma_start(out=wv_sb[:], in_=wv[:])
            bt_sb = cpool.tile([128, ST], dt.float32, tag="bt")
            nc.sync.dma_start(out=bt_sb[:], in_=bt[:])
            id_sb = cpool.tile([128, 128], dt.float32, tag="id")
            nc.sync.dma_start(out=id_sb[:], in_=ident[:])
            oc_sb = cpool.tile([128, 1], dt.float32, tag="oc")
            nc.sync.dma_start(out=oc_sb[:], in_=ones_c[:])
            or_sb = cpool.tile([1, 128], dt.float32, tag="or")
            nc.sync.dma_start(out=or_sb[:], in_=ones_r[:])

            # ~5us of junk matmuls during the load lead-in keeps the PE
            # HAM clock-gate warm for the first real score burst
            warm_ps = pm1.tile([1, 128], dt.float32, tag="m1")
            for _ in range(18):
                nc.tensor.matmul(
                    warm_ps[:], id_sb[:, 0:1], id_sb[:], start=True, stop=True
                )

            state = {}

            def stage_score(b):
                xt_sb, xn_sb = [], []
                for w in range(2):
                    t = xtpool.tile([128, DC * SH], dt.bfloat16, tag="xtw")
                    nc.sync.dma_start(
                        out=t[:], in_=xt[b, w].rearrange("p c s -> p (c s)")
                    )
                    xt_sb.append(t)
                for g in range(2):
                    t = xnpool.tile([128, (ST // 2) * D], dt.bfloat16, tag="xnw")
                    nc.sync.dma_start(
                        out=t[:], in_=xn[b, g].rearrange("p t d -> p (t d)")
                    )
                    xn_sb.append(t)

                # score = x @ W directly in [128s, ST] psum layout: the
                # xt tile [128d, 128s] is the stationary operand (M=128) and
                # the W chunk [128d, 1] streams -> psum column per s-tile.
                # FWL makes the 128-col bf16 weight loads cheap, and the
                # softmax needs no transpose roundtrip at all.
                ps128 = psc.tile([128, ST], dt.float32, tag="ps")
                for hh in range(2):
                    for si in range(16):
                        i = hh * 16 + si
                        for j in range(DC):
                            nc.tensor.matmul(
                                ps128[:, i : i + 1],
                                xt_sb[hh][:, j * SH + si * 128 : j * SH + (si + 1) * 128],
                                wv_sb[:, j : j + 1],
                                start=(j == 0),
                                stop=(j == DC - 1),
                            )
                state[b] = (xn_sb, ps128)

            def stage_post(b):
                xn_sb, ps128 = state.pop(b)
                # softmax over seq, fully parallel in [128, ST] layout
                sraw = spool.tile([128, ST], dt.float32, tag="sraw")
                nc.vector.tensor_add(sraw[:], ps128[:], bt_sb[:])
                nc.scalar.activation(sraw[:], sraw[:], AF.Tanh)
                esum = spool.tile([128, 1], dt.float32, tag="esum")
                nc.scalar.activation(sraw[:], sraw[:], AF.Exp, accum_out=esum[:])
                ps1 = pm1.tile([128, 1], dt.float32, tag="m1")
                nc.tensor.matmul(ps1[0:1, :], esum[:], oc_sb[:], start=True, stop=True)
                rec = spool.tile([1, 1], dt.float32, tag="rec")
                nc.vector.reciprocal(rec[:], ps1[0:1, :])
                prb = pm1.tile([128, 1], dt.float32, tag="m1")
                nc.tensor.matmul(prb[:], or_sb[:], rec[:], start=True, stop=True)
                rbc = spool.tile([128, 1], dt.float32, tag="rbc")
                nc.scalar.copy(rbc[:], prb[:])
                w_f = spool.tile([128, ST], dt.float32, tag="wf")
                nc.scalar.mul(w_f[:], sraw[:], rbc[:])
                w_bf = spool.tile([128, ST], dt.bfloat16, tag="wbf")
                nc.vector.tensor_copy(w_bf[:], w_f[:])

                # context = sum_s w[s] * x[s, :]
                pc = pctx.tile([1, D], dt.float32, tag="pc")
                for i in range(ST):
                    nc.tensor.matmul(
                        pc[:],
                        w_bf[:, i : i + 1],
                        xn_sb[i // 16][:, (i % 16) * D : (i % 16 + 1) * D],
                        start=(i == 0),
                        stop=(i == ST - 1),
                    )
                ctx_sb = spool.tile([1, D], dt.float32, tag="ctx")
                nc.scalar.copy(ctx_sb[:], pc[:])
                nc.gpsimd.dma_start(out=ctx_out[b : b + 1, :], in_=ctx_sb[:])

                # weights out: PE transpose -> [32, 128] -> contiguous DRAM
                pwt = pwtp.tile([ST, 128], dt.float32, tag="wt")
                nc.tensor.transpose(pwt[:], w_f[:], id_sb[:])
                wt_sb = spool.tile([ST, 128], dt.float32, tag="wts")
                nc.scalar.copy(wt_sb[:], pwt[:])
                nc.gpsimd.dma_start(
                    out=w_out[b].rearrange("(a c) -> a c", c=128), in_=wt_sb[:]
                )

            # software pipeline: emit score(b+1) before post(b) so the PE
            # queue never head-of-line blocks on batch b's softmax
            stage_score(0)
            for b in range(BPC):
                if b + 1 < BPC:
                    stage_score(b + 1)
                stage_post(b)

    if fix_waits:
        _fix_pe_waits(nc, mybir)
    return nc


def _fix_pe_waits(nc, mybir):
    """Engine instructions hold a single hardware sync-wait slot; Tile
    sometimes emits 2+ waits on one instruction (psum/tile slot reuse), which
    walrus rejects with 'Too many sync wait commands'.  Splice standalone
    EventSemaphore instructions (one wait each) into the same engine queue
    immediately before each over-subscribed instruction — semantically
    identical, the sequencer just waits in two steps."""
    f = nc.m.functions[0]
    counter = [0]
    for blk in f.blocks:
        insts = list(blk.instructions)
        out = []
        changed = False
        for inst in insts:
            si = inst.sync_info
            nw = len(si.on_wait) if si is not None and si.on_wait else 0
            if nw > 1:
                waits = list(si.on_wait)
                for w in waits[:-1]:
                    es = mybir.InstEventSemaphore(
                        name=f"I-eswait-{counter[0]}", ins=[], outs=[]
                    )
                    counter[0] += 1
                    es.engine = inst.engine
                    es.sync_info = mybir.SyncInfo(on_wait=[w], on_update=[])
                    out.append(es)
                si.on_wait = waits[-1:]
                changed = True
            out.append(inst)
        if changed:
            blk.instructions = out


def _prep_inputs(x, W, b):
    bf16 = ml_dtypes.bfloat16
    xbf = x.astype(bf16)  # (B, S, D)
    # native, seq-major halves: xn[b, g, p, t, d] = x[b, (g*16+t)*128+p, d]
    xn = np.ascontiguousarray(
        xbf.reshape(B, 2, ST // 2, 128, D).transpose(0, 1, 3, 2, 4)
    )
    # dim-major halves: xt[b, w, p, c, s] = x[b, w*2048+s, c*128+p]
    xt = np.ascontiguousarray(
        xbf.reshape(B, 2, S // 2, DC, 128).transpose(0, 1, 4, 3, 2)
    )
    wv = np.ascontiguousarray(
        W.reshape(DC, 128).T.astype(bf16)
    )  # [128, DC], col j = W[j*128:(j+1)*128]
    bt = np.ascontiguousarray(b.reshape(ST, 128).T.astype(np.float32))
    ident = np.eye(128, dtype=np.float32)
    ones_c = np.ones((128, 1), dtype=np.float32)
    ones_r = np.ones((1, 128), dtype=np.float32)
    in_maps = []
    for c in range(NCORES):
        lo = c * BPC
        in_maps.append(
            {
                "xn": xn[lo : lo + BPC],
                "xt": xt[lo : lo + BPC],
                "wv": wv,
                "bt": bt,
                "ident": ident,
                "ones_c": ones_c,
                "ones_r": ones_r,
            }
        )
    return in_maps


def kernel(x, W, b):
    from concourse.bass_utils import run_bass_kernel_spmd

    x = np.asarray(x, dtype=np.float32)
    W = np.asarray(W, dtype=np.float32)
    b = np.asarray(b, dtype=np.float32)

    if "nc" not in _cache:
        _cache["nc"] = _build()
    nc = _cache["nc"]

    in_maps = _prep_inputs(x, W, b)
    res = run_bass_kernel_spmd(nc, in_maps, list(range(NCORES))).results

    context = np.concatenate(
        [np.asarray(res[c]["ctx_out"], dtype=np.float32) for c in range(NCORES)], axis=0
    )  # (B, D)
    weights = np.concatenate(
        [np.asarray(res[c]["w_out"], dtype=np.float32) for c in range(NCORES)], axis=0
    ).reshape(B, S, 1)
    return context, weights
